# revision 12
# baseline (speedup 1.0000x reference)
"""MDN-RNN (LSTM + MDN heads) Trainium2 Bass kernel.

Sharding: data-parallel over batch B=64 -> 8 cores x 8 batch elements.
Per core:
  Phase 1: xg = W_ih @ x + b_ih + b_hh for all (t, b), written to DRAM scratch
           (bf16), computed as 8 gate-tile matmuls per 512-column chunk.
  Phase 2: recurrence over S=2048 steps in chunks of 64 steps:
           gates-on-partitions layout: PSUM G (128, 8 gate-tiles, 8 batch).
           Per step: identity-matmul injects xg chunk slice into PSUM
           (start=True), 16 accumulating matmuls add W_hh @ h_{t-1},
           ACT sigmoid/tanh -> DVE cell update -> h_t (bf16) written into the
           chunk's hs buffer (consumed directly as next step's moving operand).
           Per chunk: MDN head matmuls with hs as the stationary operand so the
           output lands (t,b)-major in PSUM -> softmax/exp postproc -> DMA out;
           hs transposed to (t,b)-major via PE transpose for the LSTM output.
"""

import sys

sys.path.insert(0, "/opt/trn_rl_repo")

import numpy as np
import ml_dtypes

import concourse.bass as bass
import concourse.tile as tile
from concourse import mybir
from concourse.bass_utils import run_bass_kernel_spmd

S, B, Z, A, H, M = 2048, 64, 32, 3, 256, 5
IN = Z + A  # 35
G4 = 4 * H  # 1024
MZ = M * Z  # 160
HD = 3 * MZ  # 480  (pi | sigma | mu)
TEMP = 1.3
NCORES = 8
BL = B // NCORES  # 8 batch elements per core
NT = S * BL  # 16384 (t, b) rows per core
CH = 512  # chunk width in (t, b) columns
TSTEPS = CH // BL  # 64 steps per chunk
NCHUNK = NT // CH  # 32

F32 = mybir.dt.float32
BF16 = mybir.dt.bfloat16
AF = mybir.ActivationFunctionType
OP = mybir.AluOpType
bf = ml_dtypes.bfloat16


def _trace(tc, nc, d):
    singles_cm = tc.tile_pool(name="singles", bufs=1)
    singles = singles_cm.__enter__()

    # --- persistent weights / constants -----------------------------------
    wih_sb = singles.tile([IN, G4], BF16)
    nc.sync.dma_start(out=wih_sb, in_=d["wihT"][:, :])
    bg_sb = singles.tile([128, 8], F32)
    nc.sync.dma_start(out=bg_sb, in_=d["bg"][:, :])
    whh_sb = singles.tile([128, 2, 8, 128], BF16)
    nc.sync.dma_start(
        out=whh_sb,
        in_=d["whhT"][:, :].rearrange("(k p) (g q) -> p k g q", p=128, q=128),
    )
    wh_sb = singles.tile([128, 2, HD], BF16)
    nc.sync.dma_start(out=wh_sb, in_=d["whT"][:, :].rearrange("(k p) n -> p k n", p=128))
    bh_sb = singles.tile([1, HD], BF16)
    nc.sync.dma_start(out=bh_sb, in_=d["bh"][:, :])
    id_sb = singles.tile([128, 128], BF16)
    nc.sync.dma_start(out=id_sb, in_=d["id128"][:, :])
    ones_sb = singles.tile([1, 128], BF16)
    nc.vector.memset(ones_sb, 1.0)

    # recurrent state: two independent batch groups of 4 (interleaved chains)
    BG = BL // 2  # 4
    cg = [singles.tile([128, 2 * BG], F32, name=f"c{i}") for i in range(2)]
    nc.vector.memset(cg[0], 0.0)
    nc.vector.memset(cg[1], 0.0)
    hsch = singles.tile([128, 2, TSTEPS, BL], BF16)  # h per chunk (persistent)
    nc.vector.memset(hsch, 0.0)

    # DRAM scratch for xg (tracked via DRAM tile pool); one chunk of slack so
    # the steady-state prefetch of the next chunk can run off the end.
    dram_cm = tc.tile_pool(name="dramxg", bufs=1, space="DRAM")
    drampool = dram_cm.__enter__()
    xg_dr = drampool.tile([8, 128, NT + CH], BF16)

    # --- Phase 1: xg precompute ------------------------------------------
    with (
        tc.tile_pool(name="p1x", bufs=3) as p1x,
        tc.tile_pool(name="p1ps", bufs=4, space="PSUM") as p1ps,
        tc.tile_pool(name="p1o", bufs=6) as p1o,
    ):
        for ch in range(NCHUNK):
            xt_sb = p1x.tile([IN, CH], BF16)
            nc.sync.dma_start(out=xt_sb, in_=d["xt"][:, ch * CH : (ch + 1) * CH])
            for gt in range(8):
                ps = p1ps.tile([128, CH], F32)
                nc.tensor.matmul(
                    ps,
                    wih_sb[:, gt * 128 : (gt + 1) * 128],
                    xt_sb,
                    start=True,
                    stop=True,
                )
                xo = p1o.tile([128, CH], BF16)
                if gt % 2 == 0:
                    nc.scalar.activation(
                        xo, ps, AF.Identity, bias=bg_sb[:, gt : gt + 1], scale=1.0
                    )
                else:
                    nc.vector.tensor_scalar_add(xo, ps, bg_sb[:, gt : gt + 1])
                nc.sync.dma_start(
                    out=xg_dr[gt, :, ch * CH : (ch + 1) * CH], in_=xo
                )

    # --- Phase 2: recurrence + heads -------------------------------------
    # Gate order is host-permuted to [i, f, o, g]: per group (BG=4 batch)
    # the G psum (128, 8 tiles, 4) flattens to (128, 32) with
    # i=0:8, f=8:16, o=16:24, g=24:32.
    with (
        tc.tile_pool(name="pg", bufs=4) as pg,
        tc.tile_pool(name="ph", bufs=2) as ph,
        tc.tile_pool(name="pho", bufs=2) as pho,
        tc.tile_pool(name="psG", bufs=4, space="PSUM") as psG,
        tc.tile_pool(name="psH", bufs=2, space="PSUM") as psH,
        tc.tile_pool(name="psT", bufs=2, space="PSUM") as psT,
    ):
        xgA = singles.tile([128, 8, CH], BF16)
        xgB = singles.tile([128, 8, CH], BF16)

        def step_mms(g, j, xgb):
            Gp = psG.tile([128, 8, BG], F32)
            nc.tensor.matmul(
                Gp,
                id_sb,
                xgb[:, :, j * BL + g * BG : j * BL + (g + 1) * BG],
                start=True,
                stop=False,
            )
            jp = (j - 1) % TSTEPS
            for gt in range(8):
                for k in range(2):
                    nc.tensor.matmul(
                        Gp[:, gt, :],
                        whh_sb[:, k, gt, :],
                        hsch[:, k, jp, g * BG : (g + 1) * BG],
                        start=False,
                        stop=(gt == 7 and k == 1),
                    )
            return Gp.rearrange("p a b -> p (a b)")  # (128, 32)

        def step_acts(Gf):
            s = pg.tile([128, 6 * BG], F32)  # sigmoid(i|f|o)
            nc.scalar.activation(s, Gf[:, 0 : 6 * BG], AF.Sigmoid)
            tg = pg.tile([128, 2 * BG], F32)  # tanh(g)
            nc.scalar.activation(tg, Gf[:, 6 * BG : 8 * BG], AF.Tanh)
            return s, tg

        def step_dves(g, s, tg):
            c = cg[g]
            t2 = pg.tile([128, 2 * BG], F32)
            nc.vector.tensor_tensor(t2, s[:, 2 * BG : 4 * BG], c, OP.mult)
            t1 = pg.tile([128, 2 * BG], F32)
            nc.vector.tensor_tensor(t1, s[:, 0 : 2 * BG], tg, OP.mult)
            nc.vector.tensor_tensor(c, t1, t2, OP.add)
            return c

        def step_tail(g, j, s, c):
            tcn = pg.tile([128, 2 * BG], F32)
            nc.scalar.activation(tcn, c, AF.Tanh)
            nc.vector.tensor_tensor(
                hsch[:, :, j, g * BG : (g + 1) * BG],
                s[:, 4 * BG : 6 * BG].rearrange("p (k b) -> p k b", k=2),
                tcn.rearrange("p (k b) -> p k b", k=2),
                OP.mult,
            )

        def heads(m, rowbase):
            hp = psH.tile([128, HD], F32)
            nc.tensor.matmul(hp, ones_sb, bh_sb, start=True, stop=False)
            for k in range(2):
                nc.tensor.matmul(
                    hp,
                    hsch[:, k, m * 16 : (m + 1) * 16, :],
                    wh_sb[:, k, :],
                    start=False,
                    stop=(k == 1),
                )
            # packed output row: [exp-pi | sigma | mu | hs] = 736 f32
            o3 = pho.tile([128, HD + H], F32)
            nc.scalar.activation(o3[:, 0 : 2 * MZ], hp[:, 0 : 2 * MZ], AF.Exp)
            nc.scalar.copy(o3[:, 2 * MZ : 3 * MZ], hp[:, 2 * MZ : 3 * MZ])
            ssum = ph.tile([128, Z], F32)
            nc.vector.tensor_reduce(
                ssum,
                o3[:, 0:MZ].rearrange("p (m z) -> p z m", m=M),
                axis=mybir.AxisListType.X,
                op=OP.add,
            )
            s2 = ph.tile([128, Z], F32)
            nc.vector.tensor_scalar_mul(s2, ssum, float(TEMP))
            rcp = ph.tile([128, Z], F32)
            nc.vector.reciprocal(rcp, s2)
            rcp_b = bass.AP(
                tensor=rcp.tensor,
                offset=rcp.offset,
                ap=[rcp.ap[0], [0, M], rcp.ap[1]],
            )
            nc.vector.tensor_tensor(
                o3[:, 0:MZ].rearrange("p (m z) -> p m z", m=M),
                o3[:, 0:MZ].rearrange("p (m z) -> p m z", m=M),
                rcp_b,
                OP.mult,
            )
            for k in range(2):
                tp = psT.tile([128, 128], BF16)
                nc.tensor.transpose(tp, hsch[:, k, m * 16 : (m + 1) * 16, :], id_sb)
                nc.scalar.copy(o3[:, HD + k * 128 : HD + (k + 1) * 128], tp)
            rows = bass.ds(rowbase + m * 128, 128)
            nc.sync.dma_start(out=d["out"][rows, :], in_=o3)

        def half(xgb, rowbase):
            for j in range(TSTEPS):
                GA = step_mms(0, j, xgb)
                GB = step_mms(1, j, xgb)
                sA, tgA = step_acts(GA)
                sB, tgB = step_acts(GB)
                cA = step_dves(0, sA, tgA)
                cB = step_dves(1, sB, tgB)
                step_tail(0, j, sA, cA)
                step_tail(1, j, sB, cB)
                if j % 16 == 15:
                    heads(j // 16, rowbase)

        hints = (
            mybir.EngineType.PE,
            mybir.EngineType.Activation,
            mybir.EngineType.DVE,
        )
        nc.sync.dma_start(
            out=xgA, in_=xg_dr[:, :, 0:CH].rearrange("g p n -> p g n")
        )
        with tc.For_i(0, NT, 2 * CH, hint_engines=hints) as iv:
            nc.sync.dma_start(
                out=xgB,
                in_=xg_dr[:, :, bass.ds(iv + CH, CH)].rearrange("g p n -> p g n"),
            )
            half(xgA, iv)
            nc.sync.dma_start(
                out=xgA,
                in_=xg_dr[:, :, bass.ds(iv + 2 * CH, CH)].rearrange("g p n -> p g n"),
            )
            half(xgB, iv + CH)

    drampool_exit = dram_cm.__exit__(None, None, None)
    singles_cm.__exit__(None, None, None)
    return drampool_exit


def _split_waits(nc, max_waits=1, max_updates=1):
    """Walrus in this container rejects instructions with more than ~1 sync
    wait; hoist extra waits onto same-engine EventSemaphore (wait-only)
    instructions placed immediately before, and spill extra updates onto
    update-only EventSemaphores immediately after (the trailing-nop pattern,
    safe per the PSUM doc: sequencer ops don't overtake engine completion
    semantics for updates emitted by Tile's clock)."""
    for f in nc.m.functions:
        for blk in f.blocks:
            out = []
            changed = False
            for inst in blk.instructions:
                si = inst.sync_info
                pre, post = [], []
                if si is not None and len(si.on_wait) > max_waits:
                    waits = list(si.on_wait)
                    extra, keep = waits[:-max_waits], waits[-max_waits:]
                    for w in extra:
                        pre.append(
                            mybir.InstEventSemaphore(
                                name=nc.get_next_instruction_name(),
                                engine=inst.engine,
                                ins=[],
                                outs=[],
                                sync_info=mybir.SyncInfo(on_wait=[w], on_update=[]),
                            )
                        )
                    si = mybir.SyncInfo(on_wait=keep, on_update=list(si.on_update))
                    inst.sync_info = si
                    changed = True
                if si is not None and len(si.on_update) > max_updates:
                    ups = list(si.on_update)
                    keep_u, extra_u = ups[:max_updates], ups[max_updates:]
                    for u in extra_u:
                        post.append(
                            mybir.InstEventSemaphore(
                                name=nc.get_next_instruction_name(),
                                engine=inst.engine,
                                ins=[],
                                outs=[],
                                sync_info=mybir.SyncInfo(on_wait=[], on_update=[u]),
                            )
                        )
                    inst.sync_info = mybir.SyncInfo(
                        on_wait=list(si.on_wait), on_update=keep_u
                    )
                    changed = True
                out.extend(pre)
                out.append(inst)
                out.extend(post)
            if changed:
                blk.instructions = out


def build():
    nc = bass.Bass("TRN2", target_bir_lowering=False, debug=False)
    d = {}
    d["xt"] = nc.declare_dram_parameter("xt", [IN, NT], BF16, isOutput=False)
    d["whhT"] = nc.declare_dram_parameter("whhT", [H, G4], BF16, isOutput=False)
    d["wihT"] = nc.declare_dram_parameter("wihT", [IN, G4], BF16, isOutput=False)
    d["bg"] = nc.declare_dram_parameter("bg", [128, 8], F32, isOutput=False)
    d["whT"] = nc.declare_dram_parameter("whT", [H, HD], BF16, isOutput=False)
    d["bh"] = nc.declare_dram_parameter("bh", [1, HD], BF16, isOutput=False)
    d["id128"] = nc.declare_dram_parameter("id128", [128, 128], BF16, isOutput=False)
    d["out"] = nc.declare_dram_parameter("out", [NT, HD + H], F32, isOutput=True)

    with tile.TileContext(nc) as tc:
        _trace(tc, nc, d)
    _split_waits(nc)
    return nc


def kernel(
    z,
    action,
    W_ih,
    W_hh,
    b_ih,
    b_hh,
    W_pi,
    b_pi,
    W_sigma,
    b_sigma,
    W_mu,
    b_mu,
    _trace_hw=False,
):
    z = np.asarray(z, np.float32)
    action = np.asarray(action, np.float32)
    W_ih = np.asarray(W_ih, np.float32)
    W_hh = np.asarray(W_hh, np.float32)
    b_ih = np.asarray(b_ih, np.float32)
    b_hh = np.asarray(b_hh, np.float32)

    x = np.concatenate([z, action], axis=-1)  # (S, B, 35)

    # permute gate blocks from PyTorch [i, f, g, o] to kernel order [i, f, o, g]
    perm = np.concatenate(
        [np.arange(0, 512), np.arange(768, 1024), np.arange(512, 768)]
    )
    W_hh = W_hh[perm]
    W_ih = W_ih[perm]
    bsum = (b_ih + b_hh)[perm]

    shared = {
        "whhT": W_hh.T.astype(bf),
        "wihT": W_ih.T.astype(bf),
        "bg": np.ascontiguousarray(bsum.reshape(8, 128).T, np.float32),
        "whT": np.concatenate(
            [np.asarray(W_pi), np.asarray(W_sigma), np.asarray(W_mu)], axis=0
        ).T.astype(bf),
        "bh": np.concatenate(
            [np.asarray(b_pi), np.asarray(b_sigma), np.asarray(b_mu)]
        ).reshape(1, HD).astype(bf),
        "id128": np.eye(128, dtype=np.float32).astype(bf),
    }
    in_maps = []
    for c in range(NCORES):
        xs = x[:, c * BL : (c + 1) * BL, :]  # (S, 8, 35)
        xt = np.ascontiguousarray(xs.transpose(2, 0, 1)).reshape(IN, NT).astype(bf)
        in_maps.append({"xt": xt, **shared})

    nc = build()
    res = run_bass_kernel_spmd(
        nc, in_maps, list(range(NCORES)), trace=_trace_hw
    )
    outs = res.results

    full = np.concatenate(
        [
            np.asarray(outs[c]["out"], np.float32).reshape(S, BL, HD + H)
            for c in range(NCORES)
        ],
        axis=1,
    )
    pi = np.ascontiguousarray(full[:, :, 0:MZ]).reshape(S, B, M, Z)
    sigma = np.ascontiguousarray(full[:, :, MZ : 2 * MZ]).reshape(S, B, M, Z)
    mu = np.ascontiguousarray(full[:, :, 2 * MZ : 3 * MZ]).reshape(S, B, M, Z)
    hs = np.ascontiguousarray(full[:, :, HD : HD + H])
    kernel._last_exec_time_ns = getattr(res, "exec_time_ns", None)
    return pi, sigma, mu, hs
